# revision 8
# baseline (speedup 1.0000x reference)
"""BitNet ternary layer on 8 trn2 NeuronCores.

y[b,s,o] = sum_i x[b,s,i] * tq(w)[o,i],  tq(w) = sign(w) * (|w| > 0.7*mean|w|)

Distribution: data-parallel over the batch dim — core c gets x[c] [2048,4096]
plus a replicated copy of the full weight [4096,4096] and the c-th 512-row
slice of it. Per core:
  A) absmean: reduce |wsl| (1/8 of w) -> per-partition partials, AllReduce the
     [128,1] partials across the 8 cores, finish the reduction locally ->
     exact global threshold t in ~60us (vs ~290us for a full-w read).
  B) x pipeline: SWDGE cast DMA x fp32 -> bf16 in DRAM, then transpose-load
     all of x^T into SBUF (stays resident: 128KB/partition).
  C) 16 output chunks of 256 columns: quantize w rows to ternary bf16 (DVE
     compares vs +/-t), round-trip through DRAM for the transpose-load of
     wq^T, then 16 s-tiles x 32 k-tiles of bf16 matmuls into PSUM. Quant runs
     2 chunks ahead and wq^T loads 1 chunk ahead so the PE never waits.
Output tiles return fp32 in natural [s, o] layout; host stacks the 8 slices.
"""
import copy
import sys

sys.path.insert(0, '/opt/trn_rl_repo')

import numpy as np

import concourse.bass as bass
from concourse import mybir
from concourse.tile import TileContext
from concourse.vector_clock import ScopedClock
from concourse.bass_utils import run_bass_kernel_spmd

# ---------------------------------------------------------------------------
# Workarounds for this container's walrus build rejecting sem-waits attached
# to several instruction structs (CTRL/NoOp/Drain/DMA-transpose): emit the
# TileContext exit waits as standalone wait_ge instructions, and post-process
# the module to hoist every immediate sem-wait onto its own single-wait
# InstEventSemaphore (same engine, same program position -> same semantics).
# ---------------------------------------------------------------------------


def _patched_drain_and_barrier(self, tick_clock, wait_clock):
    probe = self.nc.sync.nop()
    wait_clock.add_sem_waits(probe.ins, ScopedClock({None: tick_clock.global_clock}))
    si = probe.ins.sync_info
    waits = list(si.on_wait) if si is not None else []
    if waits:
        probe.ins.sync_info = mybir.SyncInfo(on_wait=[], on_update=list(si.on_update))
        for w in waits:
            self.nc.sync.wait_ge(bass.SemaphoreHandle(w.ant_name, w.id), w.wait_value)
    self.nc.sync.drain()
    self.nc.all_engine_barrier()
    assert self.sems is not None
    popped = self.nc._tile_sem_poison_stack.pop()
    assert popped is self._sem_poison
    self.nc.clear_and_free_semaphores(list(self.sems.allocated().values()))
    self.nc.all_engine_barrier()


TileContext._drain_and_barrier = _patched_drain_and_barrier

_ctr = [0]


def _hoist_waits(nc):
    new_module = copy.replace(nc.m, functions=[])
    for function in nc.m.functions:
        new_function = copy.replace(function, blocks=[])
        new_function.set_allocations_from_list(function.allocations)
        for block in function.blocks:
            new_insts = []
            for inst in block.instructions:
                si = inst.sync_info
                if si is not None and not isinstance(inst, mybir.InstEventSemaphore):
                    imm = [w for w in si.on_wait if w.wait_reg is None]
                    if imm:
                        reg = [w for w in si.on_wait if w.wait_reg is not None]
                        for w in imm:
                            _ctr[0] += 1
                            ev = mybir.InstEventSemaphore(
                                name=f"HW-{_ctr[0]}", ins=[], outs=[])
                            ev.engine = inst.engine
                            ev.sync_info = mybir.SyncInfo(on_wait=[w], on_update=[])
                            new_insts.append(ev)
                        inst.sync_info = mybir.SyncInfo(
                            on_wait=reg, on_update=list(si.on_update))
                new_insts.append(inst)
            new_block = copy.replace(block, instructions=new_insts)
            new_function.blocks.append(new_block)
        new_module.functions.append(new_function)
    nc.m = new_module
    return nc


# ---------------------------------------------------------------------------
# Problem shapes (hardcoded per spec)
# ---------------------------------------------------------------------------
B = 8            # batch -> one per core
S = 2048         # tokens per core
I = 4096         # in features (contraction)
O = 4096         # out features
P = 128
NK = I // P      # 32 k-tiles
NST = S // P     # 16 s-tiles
OC = 256         # o-chunk width
NOC = O // OC    # 16 chunks
OSL = O // B     # 512 rows of w per core for the absmean
QW = 1024        # quantize/reduce free-dim strip width
NQS = I // QW    # 4 strips per 128-row block


def build_program():
    fp32 = mybir.dt.float32
    bf16 = mybir.dt.bfloat16

    nc = bass.Bass(num_devices=B)
    x_in = nc.declare_dram_parameter("x", [S, I], fp32, isOutput=False)
    w_in = nc.declare_dram_parameter("w", [O, I], fp32, isOutput=False)
    wsl_in = nc.declare_dram_parameter("wsl", [OSL, I], fp32, isOutput=False)
    y_out = nc.declare_dram_parameter("y", [S, O], fp32, isOutput=True)

    with TileContext(nc) as tc:
        with (
            tc.tile_pool(name="dram", bufs=1, space="DRAM") as dram,
            tc.tile_pool(name="singles", bufs=1) as singles,
            tc.tile_pool(name="psum1", bufs=1, space="PSUM") as psum1,
            tc.tile_pool(name="psum", bufs=7, space="PSUM") as psum_pool,
            tc.tile_pool(name="wb", bufs=3) as wb_pool,
            tc.tile_pool(name="sl", bufs=2) as sl_pool,
            tc.tile_pool(name="qp", bufs=2) as q_pool,
            tc.tile_pool(name="wqT", bufs=2) as wqT_pool,
            tc.tile_pool(name="yo", bufs=4) as yo_pool,
        ):
            x16 = dram.tile([S, I], bf16)
            wq_oc = [dram.tile([OC, I], bf16, name=f"wq{oc}") for oc in range(NOC)]
            red = dram.tile([P, 1], fp32)
            t_dram = dram.tile([1, 1], fp32)

            xT = singles.tile([P, NST, NK, P], bf16)       # 128KB/partition
            partials = singles.tile([P, (OSL // P) * NQS], fp32)
            part1 = singles.tile([P, 1], fp32)
            redsb = singles.tile([P, 1], fp32)
            ones = singles.tile([P, 1], fp32)
            tval = singles.tile([1, 1], fp32)
            t_b = singles.tile([P, 1], fp32)
            nt_b = singles.tile([P, 1], fp32)

            # ---- helpers (defined early so the prologue can use them) ----
            def emit_quant_read(oc):
                tiles = []
                for rb in range(OC // P):
                    for cs in range(NQS):
                        wbt = wb_pool.tile([P, QW], fp32, tag="wb")
                        r0 = oc * OC + rb * P
                        nc.scalar.dma_start(
                            out=wbt[:],
                            in_=w_in[r0:r0 + P, cs * QW:(cs + 1) * QW])
                        tiles.append(wbt)
                return tiles

            # ---- x cast pipeline: fp32 -> bf16 in DRAM (SWDGE, gpsimd) ----
            for st in range(NST):
                nc.gpsimd.dma_start(
                    out=x16[st * P:(st + 1) * P, :],
                    in_=x_in[st * P:(st + 1) * P, :])

            # quant w reads for the first chunks: no deps, issue from t=0,
            # and they must precede tval in the scalar FIFO (tval blocks on
            # the collective until ~55us)
            q0_tiles = emit_quant_read(0)
            q1_tiles = emit_quant_read(1)
            q2_tiles = emit_quant_read(2)

            # ---- phase A: local slice absmean partials ----
            nc.vector.memset(ones[:], 1.0)
            for rb in range(OSL // P):
                for cs in range(NQS):
                    j = rb * NQS + cs
                    wa = sl_pool.tile([P, QW], fp32, tag="sl")
                    nc.sync.dma_start(
                        out=wa[:],
                        in_=wsl_in[rb * P:(rb + 1) * P, cs * QW:(cs + 1) * QW])
                    nc.vector.tensor_reduce(
                        partials[:, j:j + 1], wa[:],
                        axis=mybir.AxisListType.X,
                        op=mybir.AluOpType.add,
                        apply_absolute_value=True)
            nc.vector.tensor_reduce(
                part1[:], partials[:], axis=mybir.AxisListType.X,
                op=mybir.AluOpType.add)

            # ---- AllReduce the [128,1] partials across the 8 cores ----
            nc.gpsimd.dma_start(out=red[:], in_=part1[:])
            nc.gpsimd.collective_compute(
                "AllReduce", mybir.AluOpType.add,
                replica_groups=[[i for i in range(B)]],
                ins=[red[:].opt()], outs=[red[:].opt()])
            nc.gpsimd.dma_start(out=redsb[:], in_=red[:])

            # ---- finish: t = 0.7 * sum / (O*I), broadcast to [128,1] ----
            tsum = psum1.tile([1, 1], fp32)
            nc.tensor.matmul(tsum[:], lhsT=redsb[:], rhs=ones[:],
                             start=True, stop=True)
            nc.scalar.activation(tval[:], tsum[:],
                                 mybir.ActivationFunctionType.Copy,
                                 scale=0.7 / float(O * I))
            nc.gpsimd.dma_start(out=t_dram[:], in_=tval[:])
            t_bcast_ap = bass.AP(
                tensor=t_dram.tensor, offset=t_dram.offset,
                ap=[[0, P], [1, 1]])
            nc.gpsimd.dma_start(out=t_b[:], in_=t_bcast_ap)
            nc.vector.tensor_scalar_mul(nt_b[:], t_b[:], -1.0)

            # ---- more helpers ----
            def emit_quant_compute(oc, tiles):
                outs = []
                for rb in range(OC // P):
                    for cs in range(NQS):
                        wbt = tiles[rb * NQS + cs]
                        pt = q_pool.tile([P, QW], bf16, tag="pt")
                        nt = q_pool.tile([P, QW], bf16, tag="nt")
                        qt = q_pool.tile([P, QW], bf16, tag="qt")
                        nc.vector.tensor_scalar(
                            pt[:], wbt[:], t_b[:], None,
                            op0=mybir.AluOpType.is_gt)
                        nc.vector.tensor_scalar(
                            nt[:], wbt[:], nt_b[:], None,
                            op0=mybir.AluOpType.is_lt)
                        nc.vector.tensor_sub(qt[:], pt[:], nt[:])
                        outs.append(qt)
                return outs

            def emit_quant_write(oc, qts):
                for rb in range(OC // P):
                    for cs in range(NQS):
                        qt = qts[rb * NQS + cs]
                        nc.scalar.dma_start(
                            out=wq_oc[oc][rb * P:(rb + 1) * P,
                                          cs * QW:(cs + 1) * QW],
                            in_=qt[:])

            def emit_tread(oc, wqT):
                for k in range(NK):
                    nc.sync.dma_start(
                        out=wqT[:, k, :],
                        in_=wq_oc[oc][:, k * P:(k + 1) * P],
                        transpose=True)

            def emit_xt(st):
                for k in range(NK):
                    nc.sync.dma_start(
                        out=xT[:, st, k, :],
                        in_=x16[st * P:(st + 1) * P, k * P:(k + 1) * P],
                        transpose=True)

            def emit_mm(oc, wqT):
                for st in range(NST):
                    ps = psum_pool.tile([P, OC], fp32)
                    for k in range(NK):
                        nc.tensor.matmul(
                            ps[:],
                            lhsT=xT[:, st, k, :],
                            rhs=wqT[:, k, :],
                            start=(k == 0),
                            stop=(k == NK - 1))
                    ob = yo_pool.tile([P, OC], fp32)
                    nc.vector.tensor_copy(ob[:], ps[:])
                    nc.scalar.dma_start(
                        out=y_out[st * P:(st + 1) * P,
                                  oc * OC:(oc + 1) * OC],
                        in_=ob[:])

            # ---- prologue: quant chunks 0-1, first treads, xT loads ----
            q0 = emit_quant_compute(0, q0_tiles)
            q1 = emit_quant_compute(1, q1_tiles)
            emit_quant_write(0, q0)
            emit_quant_write(1, q1)

            wqT_tiles = {}
            # interleave xT loads with the first wq treads on the sync queue
            emit_xt(0)
            emit_xt(1)
            wqT_tiles[0] = wqT_pool.tile([P, NK, OC], bf16, name="wqT0", tag="wqT")
            emit_tread(0, wqT_tiles[0])
            emit_xt(2)
            emit_xt(3)
            wqT_tiles[1] = wqT_pool.tile([P, NK, OC], bf16, name="wqT1", tag="wqT")
            emit_tread(1, wqT_tiles[1])
            for st in range(4, NST):
                emit_xt(st)

            pending_read = None
            # ---- main chunk loop ----
            for oc in range(NOC):
                # quant compute for oc+2 (reads were issued one loop earlier)
                if oc + 2 < NOC:
                    if oc == 0:
                        tiles = q2_tiles
                    else:
                        tiles = pending_read
                    q = emit_quant_compute(oc + 2, tiles)
                    emit_quant_write(oc + 2, q)
                # issue reads for oc+3 (consumed next iteration)
                if oc + 3 < NOC:
                    pending_read = emit_quant_read(oc + 3)
                # transpose-load wq^T for oc+2 (quant done during chunk oc)
                if oc + 2 < NOC:
                    wqT_tiles[oc + 2] = wqT_pool.tile([P, NK, OC], bf16, name=f"wqT{oc + 2}", tag="wqT")
                    emit_tread(oc + 2, wqT_tiles[oc + 2])
                # matmuls for this chunk
                emit_mm(oc, wqT_tiles[oc])
                del wqT_tiles[oc]

    _hoist_waits(nc)
    return nc


_program_cache = None


def _get_program():
    global _program_cache
    if _program_cache is None:
        _program_cache = build_program()
    return _program_cache


def run(x, weight, trace=False):
    x = np.asarray(x, dtype=np.float32)
    weight = np.ascontiguousarray(np.asarray(weight, dtype=np.float32))
    assert x.shape == (B, S, I), x.shape
    assert weight.shape == (O, I), weight.shape
    nc = _get_program()
    in_maps = [
        {
            "x": np.ascontiguousarray(x[c]),
            "w": weight,
            "wsl": np.ascontiguousarray(weight[c * OSL:(c + 1) * OSL]),
        }
        for c in range(B)
    ]
    res = run_bass_kernel_spmd(nc, in_maps, list(range(B)), trace=trace)
    y = np.stack([res.results[c]["y"] for c in range(B)], axis=0)
    return y, res


def kernel(x, weight):
    y, _ = run(x, weight)
    return y


# revision 10
# speedup vs baseline: 1.1928x; 1.1928x over previous
"""BitNet ternary layer on 8 trn2 NeuronCores.

y[b,s,o] = sum_i x[b,s,i] * tq(w)[o,i],  tq(w) = sign(w) * (|w| > 0.7*mean|w|)

Distribution: data-parallel over the batch dim — core c gets x[c] [2048,4096]
plus a replicated copy of the full weight [4096,4096] and the c-th 512-row
slice of it. Per core:
  A) absmean: reduce |wsl| (1/8 of w) -> per-partition partials, AllReduce the
     [128,1] partials across the 8 cores, finish the reduction locally ->
     exact global threshold t in ~40us (vs ~290us for a full-w read).
  B) x pipeline: SWDGE cast DMA x fp32 -> bf16 in DRAM, then transpose-load
     all of x^T into SBUF in 4 quarter-strips (resident: 128KB/partition).
     DMA_TRANSPOSE costs ~1.2us of sync-engine time per instruction
     regardless of size, so transposes are kept large and few.
  C) 8 output chunks of 512 columns: quantize w rows to ternary bf16 (DVE
     compares vs +/-t), round-trip through DRAM for the transpose-load of
     wq^T, then 16 s-tiles x 32 k-tiles of bf16 matmuls into PSUM. Quant and
     wq^T loads run chunks ahead so the PE never waits in steady state.
Output tiles return fp32 in natural [s, o] layout; host stacks the 8 slices.
"""
import copy
import sys

sys.path.insert(0, '/opt/trn_rl_repo')

import numpy as np

import concourse.bass as bass
from concourse import mybir
from concourse.tile import TileContext
from concourse.vector_clock import ScopedClock
from concourse.bass_utils import run_bass_kernel_spmd

# ---------------------------------------------------------------------------
# Workarounds for this container's walrus build rejecting sem-waits attached
# to several instruction structs (CTRL/NoOp/Drain/DMA-transpose): emit the
# TileContext exit waits as standalone wait_ge instructions, and post-process
# the module to hoist every immediate sem-wait onto its own single-wait
# InstEventSemaphore (same engine, same program position -> same semantics).
# ---------------------------------------------------------------------------


def _patched_drain_and_barrier(self, tick_clock, wait_clock):
    probe = self.nc.sync.nop()
    wait_clock.add_sem_waits(probe.ins, ScopedClock({None: tick_clock.global_clock}))
    si = probe.ins.sync_info
    waits = list(si.on_wait) if si is not None else []
    if waits:
        probe.ins.sync_info = mybir.SyncInfo(on_wait=[], on_update=list(si.on_update))
        for w in waits:
            self.nc.sync.wait_ge(bass.SemaphoreHandle(w.ant_name, w.id), w.wait_value)
    self.nc.sync.drain()
    self.nc.all_engine_barrier()
    assert self.sems is not None
    popped = self.nc._tile_sem_poison_stack.pop()
    assert popped is self._sem_poison
    self.nc.clear_and_free_semaphores(list(self.sems.allocated().values()))
    self.nc.all_engine_barrier()


TileContext._drain_and_barrier = _patched_drain_and_barrier

_ctr = [0]


def _hoist_waits(nc):
    new_module = copy.replace(nc.m, functions=[])
    for function in nc.m.functions:
        new_function = copy.replace(function, blocks=[])
        new_function.set_allocations_from_list(function.allocations)
        for block in function.blocks:
            new_insts = []
            for inst in block.instructions:
                si = inst.sync_info
                if si is not None and not isinstance(inst, mybir.InstEventSemaphore):
                    imm = [w for w in si.on_wait if w.wait_reg is None]
                    if imm:
                        reg = [w for w in si.on_wait if w.wait_reg is not None]
                        for w in imm:
                            _ctr[0] += 1
                            ev = mybir.InstEventSemaphore(
                                name=f"HW-{_ctr[0]}", ins=[], outs=[])
                            ev.engine = inst.engine
                            ev.sync_info = mybir.SyncInfo(on_wait=[w], on_update=[])
                            new_insts.append(ev)
                        inst.sync_info = mybir.SyncInfo(
                            on_wait=reg, on_update=list(si.on_update))
                new_insts.append(inst)
            new_block = copy.replace(block, instructions=new_insts)
            new_function.blocks.append(new_block)
        new_module.functions.append(new_function)
    nc.m = new_module
    return nc


# ---------------------------------------------------------------------------
# Problem shapes (hardcoded per spec)
# ---------------------------------------------------------------------------
B = 8            # batch -> one per core
S = 2048         # tokens per core
I = 4096         # in features (contraction)
O = 4096         # out features
P = 128
NK = I // P      # 32 k-tiles
NST = S // P     # 16 s-tiles
OC = 512         # o-chunk width
NOC = O // OC    # 8 chunks
OSL = O // B     # 512 rows of w per core for the absmean
QW = 512         # quantize/reduce free-dim strip width
NQS = I // QW    # 8 strips per 128-row block
XQ = 512         # x transpose-load strip height (s rows per strip)
NXQ = S // XQ    # 4 strips


def build_program():
    fp32 = mybir.dt.float32
    bf16 = mybir.dt.bfloat16

    nc = bass.Bass(num_devices=B)
    x_in = nc.declare_dram_parameter("x", [S, I], fp32, isOutput=False)
    w_in = nc.declare_dram_parameter("w", [O, I], fp32, isOutput=False)
    wsl_in = nc.declare_dram_parameter("wsl", [OSL, I], fp32, isOutput=False)
    y_out = nc.declare_dram_parameter("y", [S, O], fp32, isOutput=True)

    with TileContext(nc) as tc:
        with (
            tc.tile_pool(name="dram", bufs=1, space="DRAM") as dram,
            tc.tile_pool(name="singles", bufs=1) as singles,
            tc.tile_pool(name="psum1", bufs=1, space="PSUM") as psum1,
            tc.tile_pool(name="psum", bufs=7, space="PSUM") as psum_pool,
            tc.tile_pool(name="wb", bufs=3) as wb_pool,
            tc.tile_pool(name="qp", bufs=2) as q_pool,
            tc.tile_pool(name="wqT", bufs=2) as wqT_pool,
            tc.tile_pool(name="yo", bufs=2) as yo_pool,
        ):
            x16 = dram.tile([S, I], bf16)
            wq_oc = [dram.tile([OC, I], bf16, name=f"wq{oc}") for oc in range(NOC)]
            red = dram.tile([P, 1], fp32)
            t_dram = dram.tile([1, 1], fp32)

            xT = singles.tile([P, NK, NST, P], bf16)       # 128KB/partition
            partials = singles.tile([P, (OSL // P) * NQS], fp32)
            part1 = singles.tile([P, 1], fp32)
            redsb = singles.tile([P, 1], fp32)
            ones = singles.tile([P, 1], fp32)
            tval = singles.tile([1, 1], fp32)
            t_b = singles.tile([P, 1], fp32)
            nt_b = singles.tile([P, 1], fp32)

            # ---- helpers ----
            def emit_quant_read(oc):
                tiles = []
                for rb in range(OC // P):
                    for cs in range(NQS):
                        wbt = wb_pool.tile([P, QW], fp32, tag="wb")
                        r0 = oc * OC + rb * P
                        nc.scalar.dma_start(
                            out=wbt[:],
                            in_=w_in[r0:r0 + P, cs * QW:(cs + 1) * QW])
                        tiles.append(wbt)
                return tiles

            def emit_quant_compute_write(oc, tiles):
                for rb in range(OC // P):
                    for cs in range(NQS):
                        wbt = tiles[rb * NQS + cs]
                        pt = q_pool.tile([P, QW], bf16, tag="pt")
                        nt = q_pool.tile([P, QW], bf16, tag="nt")
                        nc.vector.tensor_scalar(
                            pt[:], wbt[:], t_b[:], None,
                            op0=mybir.AluOpType.is_gt)
                        nc.vector.tensor_scalar(
                            nt[:], wbt[:], nt_b[:], None,
                            op0=mybir.AluOpType.is_lt)
                        nc.vector.tensor_sub(pt[:], pt[:], nt[:])
                        nc.scalar.dma_start(
                            out=wq_oc[oc][rb * P:(rb + 1) * P,
                                          cs * QW:(cs + 1) * QW],
                            in_=pt[:])

            def emit_tread(oc, wqT):
                for k in range(NK):
                    nc.sync.dma_start(
                        out=wqT[:, k, :],
                        in_=wq_oc[oc][:, k * P:(k + 1) * P],
                        transpose=True)

            def emit_xt(q):
                # one [XQ,128] -> [128,XQ] transpose per k covering s-strip q
                st0 = (q * XQ) // P
                for k in range(NK):
                    nc.sync.dma_start(
                        out=xT[:, k, st0:st0 + XQ // P, :],
                        in_=x16[q * XQ:(q + 1) * XQ, k * P:(k + 1) * P],
                        transpose=True)

            def emit_mm(oc, wqT):
                for st in range(NST):
                    ps = psum_pool.tile([P, OC], fp32)
                    for k in range(NK):
                        nc.tensor.matmul(
                            ps[:],
                            lhsT=xT[:, k, st, :],
                            rhs=wqT[:, k, :],
                            start=(k == 0),
                            stop=(k == NK - 1))
                    ob = yo_pool.tile([P, OC], fp32)
                    nc.vector.tensor_copy(ob[:], ps[:])
                    nc.scalar.dma_start(
                        out=y_out[st * P:(st + 1) * P,
                                  oc * OC:(oc + 1) * OC],
                        in_=ob[:])

            # ---- x cast pipeline: fp32 -> bf16 in DRAM (SWDGE, gpsimd) ----
            for st in range(NST):
                nc.gpsimd.dma_start(
                    out=x16[st * P:(st + 1) * P, :],
                    in_=x_in[st * P:(st + 1) * P, :])

            # ---- phase A: local slice absmean partials ----
            # (slice reads go first on the sync queue; they are done by ~20us)
            nc.vector.memset(ones[:], 1.0)
            for rb in range(OSL // P):
                for cs in range(NQS):
                    j = rb * NQS + cs
                    wa = wb_pool.tile([P, QW], fp32, tag="wb")
                    nc.sync.dma_start(
                        out=wa[:],
                        in_=wsl_in[rb * P:(rb + 1) * P, cs * QW:(cs + 1) * QW])
                    nc.vector.tensor_reduce(
                        partials[:, j:j + 1], wa[:],
                        axis=mybir.AxisListType.X,
                        op=mybir.AluOpType.add,
                        apply_absolute_value=True)
            nc.vector.tensor_reduce(
                part1[:], partials[:], axis=mybir.AxisListType.X,
                op=mybir.AluOpType.add)

            # quant w reads for chunk 0: no data deps, issue from t=0; they
            # must precede tval in the scalar FIFO (tval blocks on the
            # collective until ~35us)
            q_tiles = {0: emit_quant_read(0)}

            # ---- AllReduce the [128,1] partials across the 8 cores ----
            nc.gpsimd.dma_start(out=red[:], in_=part1[:])
            nc.gpsimd.collective_compute(
                "AllReduce", mybir.AluOpType.add,
                replica_groups=[[i for i in range(B)]],
                ins=[red[:].opt()], outs=[red[:].opt()])
            nc.gpsimd.dma_start(out=redsb[:], in_=red[:])

            # ---- finish: t = 0.7 * sum / (O*I), broadcast to [128,1] ----
            tsum = psum1.tile([1, 1], fp32)
            nc.tensor.matmul(tsum[:], lhsT=redsb[:], rhs=ones[:],
                             start=True, stop=True)
            nc.scalar.activation(tval[:], tsum[:],
                                 mybir.ActivationFunctionType.Copy,
                                 scale=0.7 / float(O * I))
            nc.gpsimd.dma_start(out=t_dram[:], in_=tval[:])
            t_bcast_ap = bass.AP(
                tensor=t_dram.tensor, offset=t_dram.offset,
                ap=[[0, P], [1, 1]])
            nc.gpsimd.dma_start(out=t_b[:], in_=t_bcast_ap)
            nc.vector.tensor_scalar_mul(nt_b[:], t_b[:], -1.0)

            # ---- prologue: prime the quant/tread/xT pipelines ----
            emit_quant_compute_write(0, q_tiles.pop(0))
            q_tiles[1] = emit_quant_read(1)

            wqT_tiles = {}
            emit_xt(0)
            wqT_tiles[0] = wqT_pool.tile([P, NK, OC], bf16, name="wqT0",
                                         tag="wqT")
            emit_tread(0, wqT_tiles[0])
            emit_xt(1)
            emit_quant_compute_write(1, q_tiles.pop(1))
            q_tiles[2] = emit_quant_read(2)
            wqT_tiles[1] = wqT_pool.tile([P, NK, OC], bf16, name="wqT1",
                                         tag="wqT")
            emit_tread(1, wqT_tiles[1])
            emit_xt(2)
            emit_xt(3)

            # ---- main chunk loop ----
            for oc in range(NOC):
                if oc + 2 < NOC:
                    emit_quant_compute_write(oc + 2, q_tiles.pop(oc + 2))
                if oc + 3 < NOC:
                    q_tiles[oc + 3] = emit_quant_read(oc + 3)
                if oc + 2 < NOC:
                    wqT_tiles[oc + 2] = wqT_pool.tile(
                        [P, NK, OC], bf16, name=f"wqT{oc + 2}", tag="wqT")
                    emit_tread(oc + 2, wqT_tiles[oc + 2])
                emit_mm(oc, wqT_tiles[oc])
                del wqT_tiles[oc]

    _hoist_waits(nc)
    return nc


_program_cache = None


def _get_program():
    global _program_cache
    if _program_cache is None:
        _program_cache = build_program()
    return _program_cache


def run(x, weight, trace=False):
    x = np.asarray(x, dtype=np.float32)
    weight = np.ascontiguousarray(np.asarray(weight, dtype=np.float32))
    assert x.shape == (B, S, I), x.shape
    assert weight.shape == (O, I), weight.shape
    nc = _get_program()
    in_maps = [
        {
            "x": np.ascontiguousarray(x[c]),
            "w": weight,
            "wsl": np.ascontiguousarray(weight[c * OSL:(c + 1) * OSL]),
        }
        for c in range(B)
    ]
    res = run_bass_kernel_spmd(nc, in_maps, list(range(B)), trace=trace)
    y = np.stack([res.results[c]["y"] for c in range(B)], axis=0)
    return y, res


def kernel(x, weight):
    y, _ = run(x, weight)
    return y


# revision 13
# speedup vs baseline: 1.3221x; 1.1084x over previous
"""BitNet ternary layer on 8 trn2 NeuronCores.

y[b,s,o] = sum_i x[b,s,i] * tq(w)[o,i],  tq(w) = sign(w) * (|w| > 0.7*mean|w|)

Distribution: data-parallel over the batch dim — core c gets x[c] [2048,4096]
plus a replicated copy of the full weight [4096,4096] and the c-th 512-row
slice of it. Per core:
  A) absmean: reduce |wsl| (1/8 of w) -> per-partition partials, AllReduce the
     [128,1] partials across the 8 cores, finish the reduction locally ->
     exact global threshold t in ~40us (vs ~290us for a full-w read).
  B) x pipeline: SWDGE cast DMA x fp32 -> bf16 in DRAM, then transpose-load
     all of x^T into SBUF in 4 quarter-strips (resident: 128KB/partition).
     DMA_TRANSPOSE costs ~1.2us of sync-engine time per instruction
     regardless of size, so transposes are kept large and few.
  C) 8 output chunks of 512 columns: quantize w rows to ternary bf16 (DVE
     compares vs +/-t), round-trip through DRAM for the transpose-load of
     wq^T, then 16 s-tiles x 32 k-tiles of bf16 matmuls into PSUM. Quant and
     wq^T loads run chunks ahead so the PE never waits in steady state.
Output tiles return fp32 in natural [s, o] layout; host stacks the 8 slices.
"""
import copy
import sys

sys.path.insert(0, '/opt/trn_rl_repo')

import numpy as np

import concourse.bass as bass
from concourse import mybir
from concourse.tile import TileContext
from concourse.vector_clock import ScopedClock
from concourse.bass_utils import run_bass_kernel_spmd

# ---------------------------------------------------------------------------
# Workarounds for this container's walrus build rejecting sem-waits attached
# to several instruction structs (CTRL/NoOp/Drain/DMA-transpose): emit the
# TileContext exit waits as standalone wait_ge instructions, and post-process
# the module to hoist every immediate sem-wait onto its own single-wait
# InstEventSemaphore (same engine, same program position -> same semantics).
# ---------------------------------------------------------------------------


def _patched_drain_and_barrier(self, tick_clock, wait_clock):
    probe = self.nc.sync.nop()
    wait_clock.add_sem_waits(probe.ins, ScopedClock({None: tick_clock.global_clock}))
    si = probe.ins.sync_info
    waits = list(si.on_wait) if si is not None else []
    if waits:
        probe.ins.sync_info = mybir.SyncInfo(on_wait=[], on_update=list(si.on_update))
        for w in waits:
            self.nc.sync.wait_ge(bass.SemaphoreHandle(w.ant_name, w.id), w.wait_value)
    self.nc.sync.drain()
    self.nc.all_engine_barrier()
    assert self.sems is not None
    popped = self.nc._tile_sem_poison_stack.pop()
    assert popped is self._sem_poison
    self.nc.clear_and_free_semaphores(list(self.sems.allocated().values()))
    self.nc.all_engine_barrier()


TileContext._drain_and_barrier = _patched_drain_and_barrier

_ctr = [0]


def _hoist_waits(nc):
    new_module = copy.replace(nc.m, functions=[])
    for function in nc.m.functions:
        new_function = copy.replace(function, blocks=[])
        new_function.set_allocations_from_list(function.allocations)
        for block in function.blocks:
            new_insts = []
            for inst in block.instructions:
                si = inst.sync_info
                if si is not None and not isinstance(inst, mybir.InstEventSemaphore):
                    imm = [w for w in si.on_wait if w.wait_reg is None]
                    if imm:
                        reg = [w for w in si.on_wait if w.wait_reg is not None]
                        for w in imm:
                            _ctr[0] += 1
                            ev = mybir.InstEventSemaphore(
                                name=f"HW-{_ctr[0]}", ins=[], outs=[])
                            ev.engine = inst.engine
                            ev.sync_info = mybir.SyncInfo(on_wait=[w], on_update=[])
                            new_insts.append(ev)
                        inst.sync_info = mybir.SyncInfo(
                            on_wait=reg, on_update=list(si.on_update))
                new_insts.append(inst)
            new_block = copy.replace(block, instructions=new_insts)
            new_function.blocks.append(new_block)
        new_module.functions.append(new_function)
    nc.m = new_module
    return nc


# ---------------------------------------------------------------------------
# Problem shapes (hardcoded per spec)
# ---------------------------------------------------------------------------
B = 8            # batch -> one per core
S = 2048         # tokens per core
I = 4096         # in features (contraction)
O = 4096         # out features
P = 128
NK = I // P      # 32 k-tiles
NST = S // P     # 16 s-tiles
OC = 512         # o-chunk width
NOC = O // OC    # 8 chunks
OSL = O // B     # 512 rows of w per core for the absmean
QW = 512         # quantize/reduce free-dim strip width
NQS = I // QW    # 8 strips per 128-row block
XQ = 512         # x transpose-load strip height (s rows per strip)
NXQ = S // XQ    # 4 strips


def build_program():
    fp32 = mybir.dt.float32
    bf16 = mybir.dt.bfloat16

    nc = bass.Bass(num_devices=B)
    x_in = nc.declare_dram_parameter("x", [S, I], fp32, isOutput=False)
    w_in = nc.declare_dram_parameter("w", [O, I], fp32, isOutput=False)
    wsl_in = nc.declare_dram_parameter("wsl", [OSL, I], fp32, isOutput=False)
    y_out = nc.declare_dram_parameter("y", [S, O], fp32, isOutput=True)

    with TileContext(nc) as tc:
        with (
            tc.tile_pool(name="dram", bufs=1, space="DRAM") as dram,
            tc.tile_pool(name="singles", bufs=1) as singles,
            tc.tile_pool(name="psum1", bufs=1, space="PSUM") as psum1,
            tc.tile_pool(name="psum", bufs=7, space="PSUM") as psum_pool,
            tc.tile_pool(name="wb", bufs=3) as wb_pool,
            tc.tile_pool(name="sl", bufs=2) as sl_pool,
            tc.tile_pool(name="qp", bufs=2) as q_pool,
            tc.tile_pool(name="qn", bufs=1) as qn_pool,
            tc.tile_pool(name="wqT", bufs=2) as wqT_pool,
            tc.tile_pool(name="yo", bufs=2) as yo_pool,
        ):
            x16 = dram.tile([S, I], bf16)
            wq_oc = [dram.tile([OC, I], bf16, name=f"wq{oc}") for oc in range(NOC)]
            red = dram.tile([P, 1], fp32)
            t_dram = dram.tile([1, 1], fp32)

            xT = singles.tile([P, NK, NST, P], bf16)       # 128KB/partition
            partials = singles.tile([P, (OSL // P) * (I // 256)], fp32)
            part1 = singles.tile([P, 1], fp32)
            redsb = singles.tile([P, 1], fp32)
            ones = singles.tile([P, 1], fp32)
            tval = singles.tile([1, 1], fp32)
            t_b = singles.tile([P, 1], fp32)
            nt_b = singles.tile([P, 1], fp32)

            # ---- helpers ----
            def emit_quant_read(oc):
                tiles = []
                for rb in range(OC // P):
                    for cs in range(NQS):
                        wbt = wb_pool.tile([P, QW], fp32, tag="wb")
                        r0 = oc * OC + rb * P
                        nc.sync.dma_start(
                            out=wbt[:],
                            in_=w_in[r0:r0 + P, cs * QW:(cs + 1) * QW])
                        tiles.append(wbt)
                return tiles

            def emit_quant_compute_write(oc, tiles):
                for rb in range(OC // P):
                    for cs in range(NQS):
                        wbt = tiles[rb * NQS + cs]
                        pt = q_pool.tile([P, QW], bf16, tag="pt")
                        nt = qn_pool.tile([P, QW], bf16, tag="nt")
                        nc.vector.tensor_scalar(
                            pt[:], wbt[:], t_b[:], None,
                            op0=mybir.AluOpType.is_gt)
                        nc.vector.tensor_scalar(
                            nt[:], wbt[:], nt_b[:], None,
                            op0=mybir.AluOpType.is_lt)
                        nc.vector.tensor_sub(pt[:], pt[:], nt[:])
                        nc.gpsimd.dma_start(
                            out=wq_oc[oc][rb * P:(rb + 1) * P,
                                          cs * QW:(cs + 1) * QW],
                            in_=pt[:])

            def emit_tread(oc, wqT):
                for k in range(NK):
                    nc.sync.dma_start(
                        out=wqT[:, k, :],
                        in_=wq_oc[oc][:, k * P:(k + 1) * P],
                        transpose=True)

            def emit_xt(q):
                # one [XQ,128] -> [128,XQ] transpose per k covering s-strip q
                st0 = (q * XQ) // P
                for k in range(NK):
                    nc.scalar.dma_start(
                        out=xT[:, k, st0:st0 + XQ // P, :],
                        in_=x16[q * XQ:(q + 1) * XQ, k * P:(k + 1) * P],
                        transpose=True)

            def emit_mm(oc, wqT):
                for st in range(NST):
                    ps = psum_pool.tile([P, OC], fp32)
                    for k in range(NK):
                        nc.tensor.matmul(
                            ps[:],
                            lhsT=xT[:, k, st, :],
                            rhs=wqT[:, k, :],
                            start=(k == 0),
                            stop=(k == NK - 1))
                    ob = yo_pool.tile([P, OC], fp32)
                    nc.vector.tensor_copy(ob[:], ps[:])
                    nc.gpsimd.dma_start(
                        out=y_out[st * P:(st + 1) * P,
                                  oc * OC:(oc + 1) * OC],
                        in_=ob[:])

            # ---- x cast pipeline: fp32 -> bf16 in DRAM (SWDGE, gpsimd) ----
            for q in range(NXQ):
                nc.gpsimd.dma_start(
                    out=x16[q * XQ:(q + 1) * XQ, :],
                    in_=x_in[q * XQ:(q + 1) * XQ, :])

            # ---- phase A: local slice absmean partials ----
            # (slice reads go first on the sync queue; they are done by ~20us)
            nc.vector.memset(ones[:], 1.0)
            SLW = 256
            for rb in range(OSL // P):
                for cs in range(I // SLW):
                    j = rb * (I // SLW) + cs
                    wa = sl_pool.tile([P, SLW], fp32, tag="sl")
                    nc.sync.dma_start(
                        out=wa[:],
                        in_=wsl_in[rb * P:(rb + 1) * P,
                                   cs * SLW:(cs + 1) * SLW])
                    nc.vector.tensor_reduce(
                        partials[:, j:j + 1], wa[:],
                        axis=mybir.AxisListType.X,
                        op=mybir.AluOpType.add,
                        apply_absolute_value=True)
            nc.vector.tensor_reduce(
                part1[:], partials[:], axis=mybir.AxisListType.X,
                op=mybir.AluOpType.add)

            # quant w reads for chunk 0: no data deps, issue right after the
            # slice reads on the sync queue
            q_tiles = {0: emit_quant_read(0)}

            # ---- AllReduce the [128,1] partials across the 8 cores ----
            nc.gpsimd.dma_start(out=red[:], in_=part1[:])
            nc.gpsimd.collective_compute(
                "AllReduce", mybir.AluOpType.add,
                replica_groups=[[i for i in range(B)]],
                ins=[red[:].opt()], outs=[red[:].opt()])
            nc.gpsimd.dma_start(out=redsb[:], in_=red[:])

            # ---- finish: t = 0.7 * sum / (O*I), broadcast to [128,1] ----
            tsum = psum1.tile([1, 1], fp32)
            nc.tensor.matmul(tsum[:], lhsT=redsb[:], rhs=ones[:],
                             start=True, stop=True)
            nc.scalar.activation(tval[:], tsum[:],
                                 mybir.ActivationFunctionType.Copy,
                                 scale=0.7 / float(O * I))
            nc.gpsimd.dma_start(out=t_dram[:], in_=tval[:])
            t_bcast_ap = bass.AP(
                tensor=t_dram.tensor, offset=t_dram.offset,
                ap=[[0, P], [1, 1]])
            nc.gpsimd.dma_start(out=t_b[:], in_=t_bcast_ap)
            nc.vector.tensor_scalar_mul(nt_b[:], t_b[:], -1.0)

            # ---- prologue: prime the quant/tread/xT pipelines ----
            # scalar queue carries ONLY tval + the 128 xT transposes, which
            # track the 4 cast strips with no other blockers.
            emit_xt(0)
            emit_xt(1)
            emit_xt(2)
            emit_xt(3)

            emit_quant_compute_write(0, q_tiles.pop(0))
            q_tiles[1] = emit_quant_read(1)

            wqT_tiles = {}
            wqT_tiles[0] = wqT_pool.tile([P, NK, OC], bf16, name="wqT0",
                                         tag="wqT")
            emit_tread(0, wqT_tiles[0])
            emit_quant_compute_write(1, q_tiles.pop(1))
            q_tiles[2] = emit_quant_read(2)
            wqT_tiles[1] = wqT_pool.tile([P, NK, OC], bf16, name="wqT1",
                                         tag="wqT")
            emit_tread(1, wqT_tiles[1])

            # ---- main chunk loop ----
            for oc in range(NOC):
                if oc + 2 < NOC:
                    emit_quant_compute_write(oc + 2, q_tiles.pop(oc + 2))
                if oc + 3 < NOC:
                    q_tiles[oc + 3] = emit_quant_read(oc + 3)
                if oc + 2 < NOC:
                    wqT_tiles[oc + 2] = wqT_pool.tile(
                        [P, NK, OC], bf16, name=f"wqT{oc + 2}", tag="wqT")
                    emit_tread(oc + 2, wqT_tiles[oc + 2])
                emit_mm(oc, wqT_tiles[oc])
                del wqT_tiles[oc]

    _hoist_waits(nc)
    return nc


_program_cache = None


def _get_program():
    global _program_cache
    if _program_cache is None:
        _program_cache = build_program()
    return _program_cache


def run(x, weight, trace=False):
    x = np.asarray(x, dtype=np.float32)
    weight = np.ascontiguousarray(np.asarray(weight, dtype=np.float32))
    assert x.shape == (B, S, I), x.shape
    assert weight.shape == (O, I), weight.shape
    nc = _get_program()
    in_maps = [
        {
            "x": np.ascontiguousarray(x[c]),
            "w": weight,
            "wsl": np.ascontiguousarray(weight[c * OSL:(c + 1) * OSL]),
        }
        for c in range(B)
    ]
    res = run_bass_kernel_spmd(nc, in_maps, list(range(B)), trace=trace)
    y = np.stack([res.results[c]["y"] for c in range(B)], axis=0)
    return y, res


def kernel(x, weight):
    y, _ = run(x, weight)
    return y


# revision 23
# speedup vs baseline: 1.4829x; 1.1216x over previous
"""BitNet ternary layer on 8 trn2 NeuronCores.

y[b,s,o] = sum_i x[b,s,i] * tq(w)[o,i],  tq(w) = sign(w) * (|w| > 0.7*mean|w|)

Distribution: data-parallel over the batch dim — core c gets x[c] [2048,4096]
and a replicated copy of the full weight [4096,4096]. Each core:
  A) reduces |w| to the global absmean -> threshold t (exact fp32 chain)
  B) casts its x slice to bf16 in DRAM (SWDGE cast DMA)
  C) quantizes w to ternary bf16 ({-1,0,+1} exact in bf16), then streams
     transpose-loads of both operands into SBUF and runs 4096 bf16 matmuls
     (K=128, M=128, N=512) accumulating over 32 k-tiles into PSUM.
Output tiles come back fp32 in the natural [s, o] layout -> host stacks the
8 per-core slices into the [8, 2048, 4096] result.
"""
import copy
import sys

sys.path.insert(0, '/opt/trn_rl_repo')

import numpy as np

import concourse.bass as bass
from concourse import mybir
from concourse.tile import TileContext
from concourse.vector_clock import ScopedClock
from concourse.bass_utils import run_bass_kernel_spmd

# ---------------------------------------------------------------------------
# Workarounds for this container's walrus build rejecting sem-waits attached
# to several instruction structs (CTRL/NoOp/Drain/DMA-transpose): emit the
# TileContext exit waits as standalone wait_ge instructions, and post-process
# the module to hoist every immediate sem-wait onto its own single-wait
# InstEventSemaphore (same engine, same program position -> same semantics).
# ---------------------------------------------------------------------------


def _patched_drain_and_barrier(self, tick_clock, wait_clock):
    probe = self.nc.sync.nop()
    wait_clock.add_sem_waits(probe.ins, ScopedClock({None: tick_clock.global_clock}))
    si = probe.ins.sync_info
    waits = list(si.on_wait) if si is not None else []
    if waits:
        probe.ins.sync_info = mybir.SyncInfo(on_wait=[], on_update=list(si.on_update))
        for w in waits:
            self.nc.sync.wait_ge(bass.SemaphoreHandle(w.ant_name, w.id), w.wait_value)
    self.nc.sync.drain()
    self.nc.all_engine_barrier()
    assert self.sems is not None
    popped = self.nc._tile_sem_poison_stack.pop()
    assert popped is self._sem_poison
    self.nc.clear_and_free_semaphores(list(self.sems.allocated().values()))
    self.nc.all_engine_barrier()


TileContext._drain_and_barrier = _patched_drain_and_barrier

_ctr = [0]


def _hoist_waits(nc):
    new_module = copy.replace(nc.m, functions=[])
    for function in nc.m.functions:
        new_function = copy.replace(function, blocks=[])
        new_function.set_allocations_from_list(function.allocations)
        for block in function.blocks:
            new_insts = []
            for inst in block.instructions:
                si = inst.sync_info
                if si is not None and not isinstance(inst, mybir.InstEventSemaphore):
                    imm = [w for w in si.on_wait if w.wait_reg is None]
                    if imm:
                        reg = [w for w in si.on_wait if w.wait_reg is not None]
                        for w in imm:
                            _ctr[0] += 1
                            ev = mybir.InstEventSemaphore(
                                name=f"HW-{_ctr[0]}", ins=[], outs=[])
                            ev.engine = inst.engine
                            ev.sync_info = mybir.SyncInfo(on_wait=[w], on_update=[])
                            new_insts.append(ev)
                        inst.sync_info = mybir.SyncInfo(
                            on_wait=reg, on_update=list(si.on_update))
                new_insts.append(inst)
            new_block = copy.replace(block, instructions=new_insts)
            new_function.blocks.append(new_block)
        new_module.functions.append(new_function)
    nc.m = new_module
    return nc


# ---------------------------------------------------------------------------
# Problem shapes (hardcoded per spec)
# ---------------------------------------------------------------------------
B = 8            # batch -> one per core
S = 2048         # tokens per core
I = 4096         # in features (contraction)
O = 4096         # out features
P = 128
NK = I // P      # 32 k-tiles
OC = 512         # o-chunk width (one PSUM bank at fp32)
NOC = O // OC    # 8
SH = 1024        # token half kept SBUF-resident as x^T
NH = S // SH     # 2 halves
NSB = SH // P    # 8 s-tiles per half
QF = 2048        # quantize free-dim chunk


def build_program(skip_a=False, skip_quant=False, skip_xpose=False,
                  skip_mm=False, skip_xcast=False, reps=1):
    fp32 = mybir.dt.float32
    bf16 = mybir.dt.bfloat16

    nc = bass.Bass(num_devices=B)
    x_in = nc.declare_dram_parameter("x", [S, I], fp32, isOutput=False)
    w_in = nc.declare_dram_parameter("w", [O, I], fp32, isOutput=False)
    wsl_in = nc.declare_dram_parameter("wsl", [O // B, I], fp32, isOutput=False)
    y_out = nc.declare_dram_parameter("y", [S, O], fp32, isOutput=True)

    with TileContext(nc) as tc:
        with (
            tc.tile_pool(name="dram", bufs=1, space="DRAM") as dram,
            tc.tile_pool(name="singles", bufs=1) as singles,
            tc.tile_pool(name="psum1", bufs=1, space="PSUM") as psum1,
            tc.tile_pool(name="psum", bufs=6, space="PSUM") as psum_pool,
            tc.tile_pool(name="outsb", bufs=4) as outsb,
        ):
            x16 = dram.tile([S, I], bf16)
            wq_oc = [dram.tile([OC, I], bf16, name=f"wq{oc}") for oc in range(NOC)]
            t_dram = dram.tile([1, 1], fp32)
            red = dram.tile([P, 1], fp32)
            partials = singles.tile([P, O // P], fp32)
            part1 = singles.tile([P, 1], fp32)
            ones = singles.tile([P, 1], fp32)
            tval = singles.tile([1, 1], fp32)
            t_b = singles.tile([P, 1], fp32)
            nt_b = singles.tile([P, 1], fp32)

            for rep in range(reps):
                # ---- Phase B: x fp32 -> bf16 in DRAM. h1 rows go via
                # SWDGE cast DMA (have time); h0 rows go through SBUF with a
                # DVE cast (HWDGE both ways, ~2x faster than the cast DMA)
                if not skip_xcast:
                    for j in range(8, 16):
                        nc.gpsimd.dma_start(
                            out=x16[j * P:(j + 1) * P, :],
                            in_=x_in[j * P:(j + 1) * P, :])

                # ---- Phase A: threshold t = 0.7 * mean|w| ----
                if skip_a:
                    nc.vector.memset(t_b[:], 0.5585)
                    nc.vector.memset(nt_b[:], -0.5585)
                if not skip_a:
                    with tc.tile_pool(name="pha", bufs=3) as pha:
                        for j in range(O // B // P):
                            wa = pha.tile([P, I], fp32, tag="wa")
                            nc.sync.dma_start(
                                out=wa[:], in_=wsl_in[j * P:(j + 1) * P, :])
                            nc.vector.tensor_reduce(
                                partials[:, j:j + 1], wa[:],
                                axis=mybir.AxisListType.X,
                                op=mybir.AluOpType.add,
                                apply_absolute_value=True)
                        for j in range(8):
                            xa = pha.tile([P, I], fp32, tag="wa")
                            nc.sync.dma_start(
                                out=xa[:], in_=x_in[j * P:(j + 1) * P, :])
                            xb = pha.tile([P, I], bf16, tag="xb")
                            nc.vector.tensor_copy(xb[:], xa[:])
                            nc.scalar.dma_start(
                                out=x16[j * P:(j + 1) * P, :], in_=xb[:])
                    nc.vector.tensor_reduce(
                        part1[:], partials[:, 0:O // B // P],
                        axis=mybir.AxisListType.X,
                        op=mybir.AluOpType.add)
                    nc.gpsimd.dma_start(out=red[:], in_=part1[:])
                    nc.gpsimd.collective_compute(
                        "AllReduce", mybir.AluOpType.add,
                        replica_groups=[[i for i in range(B)]],
                        ins=[red[:].opt()], outs=[red[:].opt()])
                    nc.gpsimd.dma_start(out=part1[:], in_=red[:])
                    nc.vector.memset(ones[:], 1.0)
                    tsum = psum1.tile([1, 1], fp32)
                    nc.tensor.matmul(tsum[:], lhsT=part1[:], rhs=ones[:],
                                     start=True, stop=True)
                    nc.scalar.activation(tval[:], tsum[:],
                                         mybir.ActivationFunctionType.Copy,
                                         scale=0.7 / float(O * I))
                    nc.sync.dma_start(out=t_dram[:], in_=tval[:])
                    t_bcast_ap = bass.AP(
                        tensor=t_dram.tensor, offset=t_dram.offset,
                        ap=[[0, P], [1, 1]])
                    nc.gpsimd.dma_start(out=t_b[:], in_=t_bcast_ap)
                    nc.vector.tensor_scalar_mul(nt_b[:], t_b[:], -1.0)

                # ---- Phase C: quantize + matmul pipeline ----
                with (
                    tc.tile_pool(name="quant", bufs=2) as quant,
                    tc.tile_pool(name="xT_pool", bufs=1) as xT_pool,
                    tc.tile_pool(name="wqT_pool", bufs=3) as wqT_pool,
                ):
                    xT = xT_pool.tile([P, NK, SH], bf16)
                    for h in range(NH):
                        if not skip_xpose:
                            for k in range(NK):
                                nc.sync.dma_start(
                                    out=xT[:, k, :],
                                    in_=x16[h * SH:(h + 1) * SH,
                                            k * P:(k + 1) * P],
                                    transpose=True)
                        for oc in range(NOC):
                            if h == 0 and not skip_quant:
                                for mb in range(OC // P):
                                    r0 = oc * OC + mb * P
                                    for cc in range(I // QF):
                                        c0 = cc * QF
                                        wb = quant.tile([P, QF], fp32,
                                                        tag="wb")
                                        nc.sync.dma_start(
                                            out=wb[:],
                                            in_=w_in[r0:r0 + P, c0:c0 + QF])
                                        pt = quant.tile([P, QF], bf16,
                                                        tag="pt")
                                        nt = quant.tile([P, QF], bf16,
                                                        tag="nt")
                                        nc.vector.tensor_scalar(
                                            pt[:], wb[:], t_b[:], None,
                                            op0=mybir.AluOpType.is_gt)
                                        nc.vector.tensor_scalar(
                                            nt[:], wb[:], nt_b[:], None,
                                            op0=mybir.AluOpType.is_lt)
                                        nc.vector.tensor_sub(pt[:], pt[:],
                                                             nt[:])
                                        nc.scalar.dma_start(
                                            out=wq_oc[oc][
                                                mb * P:(mb + 1) * P,
                                                c0:c0 + QF],
                                            in_=pt[:])
                            wqT = wqT_pool.tile([P, NK, OC], bf16)
                            if not skip_xpose:
                                for k in range(NK):
                                    nc.sync.dma_start(
                                        out=wqT[:, k, :],
                                        in_=wq_oc[oc][:, k * P:(k + 1) * P],
                                        transpose=True)
                            if not skip_mm:
                                for s in range(NSB):
                                    ps = psum_pool.tile([P, OC], fp32)
                                    for k in range(NK):
                                        nc.tensor.matmul(
                                            ps[:],
                                            lhsT=xT[:, k, s * P:(s + 1) * P],
                                            rhs=wqT[:, k, :],
                                            start=(k == 0),
                                            stop=(k == NK - 1))
                                    ob = outsb.tile([P, OC], fp32)
                                    nc.scalar.activation(
                                        ob[:], ps[:],
                                        mybir.ActivationFunctionType.Copy)
                                    nc.scalar.dma_start(
                                        out=y_out[
                                            h * SH + s * P:
                                            h * SH + (s + 1) * P,
                                            oc * OC:(oc + 1) * OC],
                                        in_=ob[:])
                if reps > 1:
                    tc.strict_bb_all_engine_barrier()

    _hoist_waits(nc)
    return nc


_program_cache = None


def _get_program():
    global _program_cache
    if _program_cache is None:
        _program_cache = build_program()
    return _program_cache


def run(x, weight, trace=False):
    x = np.asarray(x, dtype=np.float32)
    weight = np.ascontiguousarray(np.asarray(weight, dtype=np.float32))
    assert x.shape == (B, S, I), x.shape
    assert weight.shape == (O, I), weight.shape
    nc = _get_program()
    in_maps = [
        {"x": np.ascontiguousarray(x[c]), "w": weight,
         "wsl": np.ascontiguousarray(weight[c * (O // B):(c + 1) * (O // B)])}
        for c in range(B)]
    res = run_bass_kernel_spmd(nc, in_maps, list(range(B)), trace=trace)
    y = np.stack([res.results[c]["y"] for c in range(B)], axis=0)
    return y, res


def kernel(x, weight):
    y, _ = run(x, weight)
    return y

